# revision 1
# baseline (speedup 1.0000x reference)
"""Trainium2 Bass kernel for nn_AttentionBlock (B=4, C=256, H=W=64, IC=128).

Sharding: 8 cores = 4 batches x 2 row-halves of the N=4096 attention dim.
Each core computes its 2048 rows of the attention output, the final 1x1 conv
(wy), and partial BatchNorm statistics; a tiny AllReduce combines the BN
stats; each core then applies BN + residual and writes its output slice.

Algebraic simplifications vs the reference (all exact):
  - g_b and w_b only add a per-channel constant to wy, which BatchNorm's
    mean subtraction cancels -> dropped.
  - dy_b (phi bias) only adds row-constant terms to the attention logits,
    which softmax cancels -> dropped. Only dx_b (theta bias) is applied
    (folded into the theta PSUM->SBUF copy as a per-partition bias).
  - softmax is computed without max-subtraction: logits are bounded
    (|f| < ~70 for randn inputs), well within f32/bf16 exp range.

v2 structure:
  - Input DMA / f32->f16 cast / projection matmuls are chunked and
    pipelined so the TensorEngine starts ~8us in instead of ~30us.
  - The softmax denominator d[n] = sum_m exp(fT[m,n]) is accumulated on
    the GPSIMD (Pool) engine as a [128,1024] f32 running sum; only the
    final 128-partition reduction rides the PE (2x512-col ones-matmuls).
    This removes 65k PSUM columns (~27us) from the TensorEngine.
  - 1/d is broadcast across partitions with gpsimd.partition_broadcast.
  - BN statistics: sum(wy) via DVE tensor_tensor_reduce (which also
    materializes wy in SBUF), sum(wy^2) via scalar Square+accum - the two
    run on different engines in parallel.
  - BN scalar math is done for both channel groups at once ([128,2]).
"""

import sys
import numpy as np

if "/opt/trn_rl_repo" not in sys.path:
    sys.path.insert(0, "/opt/trn_rl_repo")

import concourse.bass as bass
import concourse.bacc as bacc
import concourse.mybir as mybir
import concourse.tile as tile
from concourse.bass_utils import run_bass_kernel_spmd

N_CORES = 8
B, C, HW = 4, 256, 64
N = HW * HW          # 4096 spatial positions per batch
IC = 128             # inter channels
NL = N // 2          # 2048 rows per core
NH = NL // 2         # 1024 cols per attention n-half
EPS = 1e-5
CNT = float(B * N)   # BatchNorm count per channel

f32 = mybir.dt.float32
bf16 = mybir.dt.bfloat16
f16 = mybir.dt.float16
ALU = mybir.AluOpType
ACTF = mybir.ActivationFunctionType

import os
USE_GPSIMD_D = os.environ.get("K_GPSIMD_D", "1") == "1"
USE_GPSIMD_BCAST = os.environ.get("K_GPSIMD_BCAST", "1") == "1"
USE_PIPE = os.environ.get("K_PIPE", "1") == "1"          # chunked DMA+cast+proj
# NOTE: tensor_tensor_reduce crashes the device on this runtime (probed:
# both op0=bypass and op0=add variants die) - keep stats on the scalar engine.
USE_TTR = os.environ.get("K_TTR", "0") == "1"
USE_THETA_SC = os.environ.get("K_THETA_SC", "1") == "1"  # theta bias on scalar


def _mm(nc, out, lhsT, rhs, start=True, stop=True):
    return nc.tensor.matmul(out, lhsT, rhs, start=start, stop=stop)


def _build():
    nc = bacc.Bacc("TRN2", target_bir_lowering=False, debug=False,
                   num_devices=N_CORES)

    xl_d = nc.dram_tensor("xl", [C, NL], f32, kind="ExternalInput").ap()
    yl_d = nc.dram_tensor("yl", [C, N], f32, kind="ExternalInput").ap()
    wpk_d = nc.dram_tensor("wpk", [C, 386], f32, kind="ExternalInput").ap()
    wpk2_d = nc.dram_tensor("wpk2", [IC, 257], f32, kind="ExternalInput").ap()
    out_d = nc.dram_tensor("out", [C, NL], f32, kind="ExternalOutput").ap()

    with tile.TileContext(nc) as tc:
        _emit(nc, tc, xl_d, yl_d, wpk_d, wpk2_d, out_d)
    nc.compile()
    return nc


def _emit(nc, tc, xl_d, yl_d, wpk_d, wpk2_d, out_d):
    with (
        tc.tile_pool(name="sb_w", bufs=1) as wp,        # weights + tiny tiles
        tc.tile_pool(name="sb_x", bufs=1) as xp,        # x / y staging
        tc.tile_pool(name="sb_a", bufs=1) as ap_,       # theta/phi/g activations
        tc.tile_pool(name="sb_e", bufs=6) as ep,        # exp tiles
        tc.tile_pool(name="sb_m", bufs=2) as mp,        # misc per-half tiles
        tc.tile_pool(name="sb_bn", bufs=1) as bp,       # bn tiny tiles
        tc.tile_pool(name="ps", bufs=2, space="PSUM") as pp,
        tc.tile_pool(name="dram", bufs=1, space="DRAM") as dr,
    ):
        # ---------------- weights: 3 packed DMAs on the sync queue ----------
        w1 = [wp.tile([128, 386], f32, tag=f"w1_{i}", name=f"w1_{i}")
              for i in range(2)]
        w2 = wp.tile([IC, 257], f32, tag="w2")
        for i in range(2):
            nc.sync.dma_start(w1[i][:], wpk_d[128 * i:128 * (i + 1), :])
        nc.sync.dma_start(w2[:], wpk2_d[:])
        wh1 = [wp.tile([128, 384], f16, tag=f"wh1_{i}", name=f"wh1_{i}")
               for i in range(2)]
        for i in range(2):
            nc.vector.tensor_copy(wh1[i][:], w1[i][:, 0:384])
        wdx_h = [wh1[i][:, 0:128] for i in range(2)]
        wdy_h = [wh1[i][:, 128:256] for i in range(2)]
        wg_h = [wh1[i][:, 256:384] for i in range(2)]
        gamma_t = [w1[i][:, 384:385] for i in range(2)]
        beta_t = [w1[i][:, 385:386] for i in range(2)]
        wwT_b = wp.tile([IC, C], bf16, tag="wwT_b")
        nc.vector.tensor_copy(wwT_b[:], w2[:, 0:256])
        dxb_t = wp.tile([IC, 1], f32, tag="dxb")
        nc.vector.tensor_copy(dxb_t[:], w2[:, 256:257])

        ones_m = wp.tile([128, 1], bf16, tag="ones_m")   # d-matmul stationary
        nc.vector.memset(ones_m[:], 1.0)
        ones_mf = wp.tile([128, 1], f32, tag="ones_mf")  # f32r d-reduce stationary
        nc.vector.memset(ones_mf[:], 1.0)
        ones_r = wp.tile([1, 128], f32, tag="ones_r")    # rinv bcast stationary
        nc.vector.memset(ones_r[:], 1.0)

        # ---------------- input staging tiles ----------------
        xl_t = [xp.tile([128, NL], f32, tag=f"xl{c}", bufs=1, name=f"xl{c}")
                for c in range(2)]
        xh_t = [xp.tile([128, NL], f16, tag=f"xh{c}", bufs=1, name=f"xh{c}")
                for c in range(2)]
        # y goes straight to f16 via SWDGE cast-DMA (no f32 staging)
        yh_t = [xp.tile([128, N], f16, tag=f"yh{c}", bufs=1, name=f"yh{c}")
                for c in range(2)]
        theta_h = ap_.tile([IC, NL], f16, tag="theta")
        phi_h = ap_.tile([IC, N], f16, tag="phi")
        g_sb = ap_.tile([128, N], bf16, tag="g")   # 32 chunks [m128, ic128]

        # x DMA (sync queue, t0 first) + casts (vector)
        for t in range(2):
            sl = slice(NH * t, NH * (t + 1))
            for c in range(2):
                nc.sync.dma_start(xl_t[c][:, sl],
                                  xl_d[128 * c:128 * (c + 1), sl])
        for t in range(2):
            sl = slice(NH * t, NH * (t + 1))
            for c in range(2):
                nc.vector.tensor_copy(xh_t[c][:, sl], xl_t[c][:, sl])

        def emit_y_dma(t):
            # f32->f16 cast-DMA on the SWDGE (gpsimd) queue; emitted in-loop
            # so engine order provides prefetch timing.
            sl = slice(1024 * t, 1024 * (t + 1))
            for c in range(2):
                nc.gpsimd.dma_start(yh_t[c][:, sl],
                                    yl_d[128 * c:128 * (c + 1), sl])

        def emit_theta(t):
            ssl = slice(NH * t, NH * (t + 1))
            tp = pp.tile([128, 1024], f32, tag="q", name=f"thp{t}")
            for c in range(2):
                for j in range(2):
                    _mm(nc, tp[:, 512 * j:512 * (j + 1)], wdx_h[c],
                        xh_t[c][:, ssl.start + 512 * j:
                                  ssl.start + 512 * (j + 1)],
                        start=(c == 0), stop=(c == 1))
            if USE_THETA_SC:
                nc.scalar.activation(theta_h[:, ssl], tp[:], ACTF.Identity,
                                     bias=dxb_t[:])
            else:
                nc.vector.tensor_scalar(theta_h[:, ssl], tp[:], dxb_t[:],
                                        None, ALU.add)

        def emit_phig(t):
            # phi + g projections for y chunk t (PE + copies on vector/scalar)
            ssl = slice(1024 * t, 1024 * (t + 1))
            php = pp.tile([128, 1024], f32, tag="q", name=f"php{t}")
            for c in range(2):
                for j in range(2):
                    _mm(nc, php[:, 512 * j:512 * (j + 1)], wdy_h[c],
                        yh_t[c][:, ssl.start + 512 * j:
                                  ssl.start + 512 * (j + 1)],
                        start=(c == 0), stop=(c == 1))
            nc.vector.tensor_copy(phi_h[:, ssl], php[:])
            gp = pp.tile([128, 1024], f32, tag="q", name=f"gp{t}")
            for j in range(8):
                m = ssl.start // 128 + j
                for c in range(2):
                    _mm(nc, gp[:, 128 * j:128 * (j + 1)],
                        yh_t[c][:, 128 * m:128 * (m + 1)], wg_h[c],
                        start=(c == 0), stop=(c == 1))
            nc.scalar.copy(g_sb[:, ssl], gp[:])

        emit_y_dma(0)
        emit_theta(0)
        emit_phig(0)

        # ---------------- attention (cross-half software pipelined) --------
        wy_sb = [mp.tile([128, NL], f16, tag=f"wy{c}", bufs=1, name=f"wy_sb{c}")
                 for c in range(2)]
        # packed stats columns: [c0_sum, c0_sq, c1_sum, c1_sq] per half
        packed = [bp.tile([128, 4], f32, tag=f"packed{h}", name=f"packed{h}")
                  for h in range(2)]
        sums_sc = bp.tile([128, 16], f32, tag="sums_sc")   # scratch columns
        f32r = mybir.dt.float32r
        H = {}      # per-half state
        gstate = {}

        def begin_half(h2):
            s = {}
            s["n0"] = NH * h2
            s["y2"] = [pp.tile([IC, 512], f32, tag=f"y2_{h2}", bufs=2,
                               name=f"y2p{h2}_{j}") for j in range(2)]
            s["dacc_v"] = mp.tile([128, NH], f32r, tag="daccv", bufs=2,
                                  name=f"daccv{h2}")
            s["dacc_g"] = mp.tile([128, NH], f32r, tag="daccg", bufs=2,
                                  name=f"daccg{h2}")
            H[h2] = s
            s["ft"] = emit_f(h2, 0)

        def emit_f(h2, m):
            ft = pp.tile([128, 1024], f32, tag="q", name=f"ft{h2}_{m}")
            for j in range(2):
                _mm(nc, ft[:, 512 * j:512 * (j + 1)],
                    phi_h[:, 128 * m:128 * (m + 1)],
                    theta_h[:, H[h2]["n0"] + 512 * j:
                            H[h2]["n0"] + 512 * (j + 1)])
            return ft

        def emit_iter(h2, m):
            s = H[h2]
            expP = ep.tile([128, 1024], bf16, tag="exp", name=f"ex{h2}_{m}")
            nc.scalar.activation(expP[:], s["ft"][:], ACTF.Exp)
            if h2 == 0:
                if m == 8:
                    emit_theta(1)
                if m in (4, 12, 20):
                    emit_phig(m // 8 + 1)
            if m < 31:
                s["ft"] = emit_f(h2, m + 1)
            for j in range(2):
                _mm(nc, s["y2"][j][:], g_sb[:, 128 * m:128 * (m + 1)],
                    expP[:, 512 * j:512 * (j + 1)],
                    start=(m == 0), stop=(m == 31))
            # 3:2 DVE:gpsimd split - DVE adds run ~1.87us, gpsimd ~2.8us,
            # so an even split leaves gpsimd pacing the loop at 1.4us/iter;
            # 3-of-5 on DVE balances both at ~1.12us/iter (the exp floor).
            use_v = (m % 5) < 3
            eng = nc.vector if use_v else nc.gpsimd
            acc = s["dacc_v"] if use_v else s["dacc_g"]
            key = "v_started" if use_v else "g_started"
            if not s.get(key):
                eng.tensor_copy(acc[:], expP[:])
                s[key] = True
            else:
                eng.tensor_tensor(acc[:], acc[:], expP[:], op=ALU.add)
            if h2 == 0 and m in (2, 10, 18):
                emit_y_dma(m // 8 + 1)

        def emit_dq(h2):
            # d = colsum(dacc_g) + colsum(dacc_v) via accumulated f32r
            # ones-matmuls; then 1/d on DVE.
            s = H[h2]
            dq = pp.tile([128, 1024], f32, tag="q", name=f"dq{h2}")
            for a, acc in enumerate((s["dacc_g"], s["dacc_v"])):
                for j in range(2):
                    _mm(nc, dq[0:1, 512 * j:512 * (j + 1)],
                        ones_mf[:].bitcast(f32r),
                        acc[:, 512 * j:512 * (j + 1)],
                        start=(a == 0), stop=(a == 1))
            rinv = mp.tile([1, NH], f32, tag="rinv", name=f"ri{h2}")
            nc.vector.reciprocal_approx_fast(rinv[:], dq[0:1, :])
            s["rinv"] = rinv

        def emit_norm_wy(h2):
            s = H[h2]
            n0 = s["n0"]
            rb_sb = mp.tile([128, NH], f32, tag="rb", name=f"rb{h2}")
            if USE_GPSIMD_BCAST:
                nc.gpsimd.partition_broadcast(rb_sb[:], s["rinv"][:])
            else:
                rbq = pp.tile([128, 1024], f32, tag="q", name=f"rbq{h2}")
                for j in range(2):
                    _mm(nc, rbq[:, 512 * j:512 * (j + 1)], ones_r[:],
                        s["rinv"][:, 512 * j:512 * (j + 1)])
                nc.vector.tensor_copy(rb_sb[:], rbq[:])
            y2sb = mp.tile([IC, NH], bf16, tag="y2sb", name=f"y2sb{h2}")
            for j in range(2):
                jsl = slice(512 * j, 512 * (j + 1))
                nc.vector.tensor_tensor(y2sb[:, jsl], s["y2"][j][:],
                                        rb_sb[:, jsl], op=ALU.mult)
            # wy: 4 psum tiles reusing this half's y2 banks (freed by the
            # normalize above); stats per 512-chunk: scalar Copy+accum
            # (materializes wy in SBUF) and Square+accum.
            for c in range(2):
                base = 8 * h2 + 4 * c
                for j in range(2):
                    jsl = slice(512 * j, 512 * (j + 1))
                    wyp = pp.tile([128, 512], f32, tag=f"y2_{h2}", bufs=2,
                                  name=f"wyp{h2}_{c}_{j}")
                    _mm(nc, wyp[:], wwT_b[:, 128 * c:128 * (c + 1)],
                        y2sb[:, jsl])
                    nc.scalar.activation(
                        wy_sb[c][:, n0 + 512 * j:n0 + 512 * (j + 1)],
                        wyp[:], ACTF.Copy,
                        accum_out=sums_sc[:, base + j:base + j + 1])
                    sq = ep.tile([128, 512], f16, tag="sqscratch", bufs=2,
                                 name=f"sq{h2}_{c}_{j}")
                    nc.scalar.activation(sq[:], wyp[:], ACTF.Square,
                                         accum_out=sums_sc[:, base + j + 2:
                                                           base + j + 3])
                for t, off in ((0, 0), (1, 2)):
                    nc.vector.tensor_tensor(
                        packed[h2][:, 2 * c + t:2 * c + t + 1],
                        sums_sc[:, base + off:base + off + 1],
                        sums_sc[:, base + off + 1:base + off + 2],
                        op=ALU.add)

        def emit_ar(h2):
            ar_in = dr.tile([128, 4], f32, name=f"ar_in{h2}")
            ar_out = dr.tile([128, 4], f32, name=f"ar_out{h2}")
            # staging DMAs ride the idle sync queue - a gpsimd dma here
            # would block the d-accumulate stream behind AR completion
            nc.sync.dma_start(ar_in[:], packed[h2][:])
            nc.gpsimd.collective_compute(
                "AllReduce", ALU.add,
                replica_groups=[list(range(N_CORES))],
                ins=[ar_in.opt()], outs=[ar_out.opt()])
            gsb = bp.tile([128, 4], f32, tag=f"gsb{h2}", name=f"gsb{h2}")
            nc.sync.dma_start(gsb[:], ar_out[:])
            gstate[h2] = gsb

        K = 8   # h1 iterations emitted before h0's tail chain
        begin_half(0)
        for m in range(32):
            emit_iter(0, m)
        begin_half(1)
        emit_iter(1, 0)
        emit_iter(1, 1)
        emit_dq(0)
        for m in range(2, K):
            emit_iter(1, m)
        emit_norm_wy(0)
        for m in range(K, 12):
            emit_iter(1, m)
        emit_ar(0)
        for m in range(12, 32):
            emit_iter(1, m)
        emit_dq(1)
        emit_norm_wy(1)
        emit_ar(1)

        stats_g = bp.tile([128, 4], f32, tag="stats_g")
        nc.vector.tensor_tensor(stats_g[:], gstate[0][:], gstate[1][:],
                                op=ALU.add)

        # ---------------- BN math + apply + residual ----------------
        for c in range(2):
            mean = bp.tile([128, 1], f32, tag=f"mean{c}")
            nc.vector.tensor_scalar(mean[:], stats_g[:, 2 * c:2 * c + 1],
                                    1.0 / CNT, None, ALU.mult)
            msq = bp.tile([128, 1], f32, tag=f"msq{c}")
            nc.vector.tensor_scalar(msq[:], stats_g[:, 2 * c + 1:2 * c + 2],
                                    1.0 / CNT, None, ALU.mult)
            m2 = bp.tile([128, 1], f32, tag=f"m2{c}")
            nc.vector.tensor_tensor(m2[:], mean[:], mean[:], op=ALU.mult)
            var = bp.tile([128, 1], f32, tag=f"var{c}")
            nc.vector.tensor_tensor(var[:], msq[:], m2[:], op=ALU.subtract)
            varep = bp.tile([128, 1], f32, tag=f"varep{c}")
            nc.vector.tensor_scalar(varep[:], var[:], float(EPS), None, ALU.add)
            sd = bp.tile([128, 1], f32, tag=f"sd{c}")
            nc.scalar.activation(sd[:], varep[:], ACTF.Sqrt)
            rstd = bp.tile([128, 1], f32, tag=f"rstd{c}")
            nc.vector.reciprocal(rstd[:], sd[:])
            scale = bp.tile([128, 1], f32, tag=f"scale{c}")
            nc.vector.tensor_tensor(scale[:], gamma_t[c], rstd[:], op=ALU.mult)
            msc = bp.tile([128, 1], f32, tag=f"msc{c}")
            nc.vector.tensor_tensor(msc[:], mean[:], scale[:], op=ALU.mult)
            shift = bp.tile([128, 1], f32, tag=f"shift{c}")
            nc.vector.tensor_tensor(shift[:], beta_t[c], msc[:], op=ALU.subtract)

            out_t = mp.tile([128, NL], f32, tag=f"out{c}", bufs=1,
                            name=f"out{c}")
            for k in range(2):
                sl = slice(1024 * k, 1024 * (k + 1))
                nc.vector.affine_then_add(out_t[:, sl], wy_sb[c][:, sl],
                                          xl_t[c][:, sl], scale[:], shift[:])
                nc.sync.dma_start(out_d[128 * c:128 * (c + 1), sl],
                                  out_t[:, sl])


_NC_CACHE = None


def _get_nc():
    global _NC_CACHE
    if _NC_CACHE is None:
        _NC_CACHE = _build()
    return _NC_CACHE


def shard_inputs(inputs):
    x = np.ascontiguousarray(inputs["x"], dtype=np.float32).reshape(B, C, N)
    y = np.ascontiguousarray(inputs["y"], dtype=np.float32).reshape(B, C, N)
    dxwT = np.asarray(inputs["dx_w"]).T.astype(np.float32)
    dywT = np.asarray(inputs["dy_w"]).T.astype(np.float32)
    gwT = np.asarray(inputs["g_w"]).T.astype(np.float32)
    wwT = np.asarray(inputs["w_w"]).T.astype(np.float32)
    dxb = np.asarray(inputs["dx_b"], dtype=np.float32).reshape(IC, 1)
    gamma = np.asarray(inputs["bn_gamma"], dtype=np.float32).reshape(C, 1)
    beta = np.asarray(inputs["bn_beta"], dtype=np.float32).reshape(C, 1)
    # pack all small weights into two tensors (3 DMAs instead of 12)
    wpk = np.ascontiguousarray(
        np.concatenate([dxwT, dywT, gwT, gamma, beta], axis=1))   # [256, 386]
    wpk2 = np.ascontiguousarray(
        np.concatenate([wwT, dxb], axis=1))                        # [128, 257]

    in_maps = []
    for core in range(N_CORES):
        b, h = divmod(core, 2)
        in_maps.append({
            "xl": np.ascontiguousarray(x[b][:, h * NL:(h + 1) * NL]),
            "yl": y[b],
            "wpk": wpk, "wpk2": wpk2,
        })
    return in_maps


def run(inputs, **kw):
    """Run on hardware; returns (full_output, BassKernelResults)."""
    nc = _get_nc()
    in_maps = shard_inputs(inputs)
    r = run_bass_kernel_spmd(nc, in_maps, core_ids=list(range(N_CORES)), **kw)
    out = np.empty((B, C, N), np.float32)
    for core in range(N_CORES):
        b, h = divmod(core, 2)
        out[b][:, h * NL:(h + 1) * NL] = r.results[core]["out"]
    return out.reshape(B, C, HW, HW), r


def kernel(**inputs):
    out, _ = run(inputs)
    return out



# revision 13
# speedup vs baseline: 1.0966x; 1.0966x over previous
"""Trainium2 Bass kernel for nn_AttentionBlock (B=4, C=256, H=W=64, IC=128).

Sharding: 8 cores = 4 batches x 2 row-halves of the N=4096 attention dim.
Each core computes its 2048 rows of the attention output, the final 1x1 conv
(wy), and partial BatchNorm statistics; one tiny AllReduce at the end
combines the BN stats; each core then applies BN + residual and writes its
output slice.

Algebraic simplifications vs the reference (all exact):
  - g_b and w_b only add a per-channel constant to wy, which BatchNorm's
    mean subtraction cancels -> dropped.
  - dy_b (phi bias) only adds row-constant terms to the attention logits,
    which softmax cancels -> dropped. Only dx_b (theta bias) is applied.
  - softmax computed without max-subtraction: logits are bounded
    (|f| < ~70 for randn inputs), within bf16 exp range.
  - BN linear sums computed as wwT.T @ sum_n(y2norm) instead of summing wy
    (mathematically identical, frees the scalar engine).

v3 structure (vs the 209us v2 baseline):
  - Softmax denominator accumulated in bf16 (2x DVE mode) with three
    accumulators: 3-of-4 adds on DVE, 1-of-4 on Pool (Pool cannot touch
    PSUM, so it gets SBUF-only work); final column-reduce via 6 bf16
    ones-matmuls on the PE. Cuts ~90us of combined DVE+Pool time vs the
    f32 splits.
  - The scalar engine runs ONLY exp in the main loop (the 64x ~1.0us
    exp instructions are the pace floor).
  - Projections (theta/phi/g) are emitted in fine-grained blocks
    interleaved one per iteration into the h0 loop, keeping the PE stream
    dense (p-state ramp) with no big bursts.
  - rinv partition-broadcast via rank-1 f32r PE matmul (gpsimd
    partition_broadcast took 4.4us each).
  - BN linear sums via wwT.T @ reduce(y2norm) on DVE+PE instead of 8
    scalar-engine copy-accumulates.
  - Single AllReduce of [128,4] BN stats at the end.
"""

import os
import sys
import numpy as np

if "/opt/trn_rl_repo" not in sys.path:
    sys.path.insert(0, "/opt/trn_rl_repo")

import concourse.bass as bass
import concourse.bacc as bacc
import concourse.mybir as mybir
import concourse.tile as tile
from concourse.bass_utils import run_bass_kernel_spmd

N_CORES = 8
B, C, HW = 4, 256, 64
N = HW * HW          # 4096 spatial positions per batch
IC = 128             # inter channels
NL = N // 2          # 2048 rows per core
NH = NL // 2         # 1024 rows per attention half
EPS = 1e-5
CNT = float(B * N)   # BatchNorm count per channel

f32 = mybir.dt.float32
f32r = mybir.dt.float32r
bf16 = mybir.dt.bfloat16
f16 = mybir.dt.float16
ALU = mybir.AluOpType
ACTF = mybir.ActivationFunctionType

# fallback flags (1 = new behavior)
DACC16 = os.environ.get("K_DACC16", "1") == "1"       # bf16 d-accumulators


def _mm(nc, out, lhsT, rhs, start=True, stop=True):
    return nc.tensor.matmul(out, lhsT, rhs, start=start, stop=stop)


def _build():
    nc = bacc.Bacc("TRN2", target_bir_lowering=False, debug=False,
                   num_devices=N_CORES)

    xl_d = nc.dram_tensor("xl", [C, NL], f32, kind="ExternalInput").ap()
    yl_d = nc.dram_tensor("yl", [C, N], f32, kind="ExternalInput").ap()
    wpk_d = nc.dram_tensor("wpk", [C, 386], f32, kind="ExternalInput").ap()
    wpk2_d = nc.dram_tensor("wpk2", [IC, 257], f32, kind="ExternalInput").ap()
    out_d = nc.dram_tensor("out", [C, NL], f32, kind="ExternalOutput").ap()

    with tile.TileContext(nc) as tc:
        _emit(nc, tc, xl_d, yl_d, wpk_d, wpk2_d, out_d)
    nc.compile()
    return nc


def _emit(nc, tc, xl_d, yl_d, wpk_d, wpk2_d, out_d):
    DT_ACC = bf16 if DACC16 else f32r
    with (
        tc.tile_pool(name="sb_w", bufs=1) as wp,        # weights + tiny tiles
        tc.tile_pool(name="sb_x", bufs=1) as xp,        # x / y staging
        tc.tile_pool(name="sb_a", bufs=1) as ap_,       # theta/phi/g activations
        tc.tile_pool(name="sb_e", bufs=6) as ep,        # exp tiles
        tc.tile_pool(name="sb_m", bufs=2) as mp,        # misc per-half tiles
        tc.tile_pool(name="sb_bn", bufs=1) as bp,       # bn tiny tiles
        tc.tile_pool(name="ps_q", bufs=2, space="PSUM") as pq,    # ft/dq/rb
        tc.tile_pool(name="ps_a", bufs=2, space="PSUM") as pa,    # y2 h0 / wyp
        tc.tile_pool(name="ps_b", bufs=2, space="PSUM") as pb,    # proj / y2 h1
        tc.tile_pool(name="dram", bufs=1, space="DRAM") as dr,
    ):
        # ---------------- input DMAs first (x t0 -> weights -> x t1) -------
        xl_t = [xp.tile([128, NL], f32, tag=f"xl{c}", bufs=1, name=f"xl{c}")
                for c in range(2)]
        for c in range(2):
            nc.sync.dma_start(xl_t[c][:, 0:NH], xl_d[128 * c:128 * (c + 1), 0:NH])
        w1 = [wp.tile([128, 386], f32, tag=f"w1_{i}", name=f"w1_{i}")
              for i in range(2)]
        w2 = wp.tile([IC, 257], f32, tag="w2")
        for i in range(2):
            nc.sync.dma_start(w1[i][:], wpk_d[128 * i:128 * (i + 1), :])
        nc.sync.dma_start(w2[:], wpk2_d[:])
        for c in range(2):
            nc.sync.dma_start(xl_t[c][:, NH:NL], xl_d[128 * c:128 * (c + 1), NH:NL])

        # y: straight to f16 via SWDGE cast-DMA (chunks 0,1 now; 2,3 in-loop)
        yh_t = [xp.tile([128, N], f16, tag=f"yh{c}", bufs=1, name=f"yh{c}")
                for c in range(2)]

        def emit_y_dma(t):
            sl = slice(1024 * t, 1024 * (t + 1))
            for c in range(2):
                nc.gpsimd.dma_start(yh_t[c][:, sl],
                                    yl_d[128 * c:128 * (c + 1), sl])

        emit_y_dma(0)
        emit_y_dma(1)

        # ---------------- weight casts (DVE) ----------------
        wh1 = [wp.tile([128, 384], f16, tag=f"wh1_{i}", name=f"wh1_{i}")
               for i in range(2)]
        for i in range(2):
            nc.vector.tensor_copy(wh1[i][:], w1[i][:, 0:384])
        wdx_h = [wh1[i][:, 0:128] for i in range(2)]
        wdy_h = [wh1[i][:, 128:256] for i in range(2)]
        wg_h = [wh1[i][:, 256:384] for i in range(2)]
        gamma_t = [w1[i][:, 384:385] for i in range(2)]
        beta_t = [w1[i][:, 385:386] for i in range(2)]

        # x cast t0 (theta t0 needs it); t1 cast after
        xh_t = [xp.tile([128, NL], f16, tag=f"xh{c}", bufs=1, name=f"xh{c}")
                for c in range(2)]
        for c in range(2):
            nc.vector.tensor_copy(xh_t[c][:, 0:NH], xl_t[c][:, 0:NH])

        wwT_b = wp.tile([IC, C], bf16, tag="wwT_b")
        nc.vector.tensor_copy(wwT_b[:], w2[:, 0:256])
        dxb_t = wp.tile([IC, 1], f32, tag="dxb")
        nc.vector.tensor_copy(dxb_t[:], w2[:, 256:257])

        ones_mb = wp.tile([128, 1], bf16, tag="ones_mb")  # d-reduce stationary
        nc.vector.memset(ones_mb[:], 1.0)
        ones_mf = wp.tile([128, 1], f32, tag="ones_mf")
        nc.vector.memset(ones_mf[:], 1.0)
        ones_r = wp.tile([1, 128], f32, tag="ones_r")     # rinv bcast stationary
        nc.vector.memset(ones_r[:], 1.0)

        for c in range(2):
            nc.vector.tensor_copy(xh_t[c][:, NH:NL], xl_t[c][:, NH:NL])

        # ---------------- projection targets ----------------
        theta_h = ap_.tile([IC, NL], f16, tag="theta")
        phi_h = ap_.tile([IC, N], f16, tag="phi")
        g_sb = ap_.tile([128, N], bf16, tag="g")   # 32 chunks [m128, ic128]

        def emit_theta_block(tb):  # tb 0..3 : theta n-cols 512*tb
            sl = slice(512 * tb, 512 * (tb + 1))
            tp_ = pb.tile([128, 512], f32, tag="pj", name=f"thp{tb}")
            for c in range(2):
                _mm(nc, tp_[:], wdx_h[c], xh_t[c][:, sl],
                    start=(c == 0), stop=(c == 1))
            nc.vector.tensor_scalar(theta_h[:, sl], tp_[:], dxb_t[:],
                                    None, ALU.add)

        def emit_phi_block(pbk):  # pbk 0..7 : phi m-cols 512*pbk
            sl = slice(512 * pbk, 512 * (pbk + 1))
            pp_ = pb.tile([128, 512], f32, tag="pj", name=f"php{pbk}")
            for c in range(2):
                _mm(nc, pp_[:], wdy_h[c], yh_t[c][:, sl],
                    start=(c == 0), stop=(c == 1))
            nc.vector.tensor_copy(phi_h[:, sl], pp_[:])

        def emit_g_block(gbk):  # gbk 0..7 : m-chunks 4*gbk .. 4*gbk+3
            gp_ = pb.tile([128, 512], f32, tag="pj", name=f"gp{gbk}")
            for jj in range(4):
                m = 4 * gbk + jj
                for c in range(2):
                    _mm(nc, gp_[:, 128 * jj:128 * (jj + 1)],
                        yh_t[c][:, 128 * m:128 * (m + 1)], wg_h[c],
                        start=(c == 0), stop=(c == 1))
            nc.vector.tensor_copy(g_sb[:, 512 * gbk:512 * (gbk + 1)], gp_[:])

        # ---------------- per-half state ----------------
        wy_sb = [mp.tile([128, NL], f16, tag=f"wy{c}", bufs=1, name=f"wy_sb{c}")
                 for c in range(2)]
        H = {}
        sq_sum = {}    # (h, c) -> [128,1] f32 sum of wy^2
        slin = {}      # h -> [128,1] f32 sum_n y2norm

        def begin_half(h):
            s = {}
            s["n0"] = NH * h
            pool = pa if h == 0 else pb
            tag = "y2a" if h == 0 else "pj"
            s["y2"] = [pool.tile([IC, 512], f32, tag=tag, bufs=2,
                                 name=f"y2p{h}_{j}") for j in range(2)]
            s["dacc"] = [mp.tile([128, NH], DT_ACC, tag=f"dacc{a}", bufs=2,
                                 name=f"dacc{a}_{h}") for a in range(3)]
            s["dst"] = [False, False, False]
            H[h] = s
            s["ft"] = emit_f(h, 0)

        def emit_f(h, m):
            ft = pq.tile([128, 1024], f32, tag="q", name=f"ft{h}_{m}")
            n0 = H[h]["n0"]
            for j in range(2):
                _mm(nc, ft[:, 512 * j:512 * (j + 1)],
                    phi_h[:, 128 * m:128 * (m + 1)],
                    theta_h[:, n0 + 512 * j:n0 + 512 * (j + 1)])
            return ft

        def emit_dq(h):
            # d[n] = colsum over m of exp; dacc tiles hold partial sums in
            # DT_ACC; ones-matmul into [1, NH] psum (rides a q-pool tile).
            s = H[h]
            dqt = pq.tile([128, 1024], f32, tag="q", name=f"dq{h}")
            for j in range(2):
                jsl = slice(512 * j, 512 * (j + 1))
                for a in range(3):
                    rhs = s["dacc"][a][:, jsl]
                    lhs = ones_mb[:] if DACC16 else ones_mf[:].bitcast(f32r)
                    _mm(nc, dqt[0:1, jsl], lhs, rhs,
                        start=(a == 0), stop=(a == 2))
            rinv = mp.tile([1, NH], f32, tag="rinv", bufs=2, name=f"ri{h}")
            nc.vector.reciprocal_approx_fast(rinv[:], dqt[0:1, :])
            s["rinv"] = rinv

        def emit_rb(h):
            # broadcast rinv across partitions: rank-1 f32r matmul, then
            # stage to SBUF (DVE reads one PSUM operand max per op)
            s = H[h]
            rbq = pq.tile([128, 1024], f32, tag="q", name=f"rbq{h}")
            for j in range(2):
                jsl = slice(512 * j, 512 * (j + 1))
                _mm(nc, rbq[:, jsl], ones_r[:], s["rinv"][:, jsl])
            rb_sb = mp.tile([128, NH], f32, tag="rb", bufs=2, name=f"rb{h}")
            nc.vector.tensor_copy(rb_sb[:], rbq[:])
            s["rb"] = rb_sb

        def emit_y2norm(h):
            s = H[h]
            y2sb = mp.tile([IC, NH], bf16, tag="y2sb", bufs=2, name=f"y2sb{h}")
            for j in range(2):
                jsl = slice(512 * j, 512 * (j + 1))
                nc.vector.tensor_tensor(y2sb[:, jsl], s["y2"][j][:],
                                        s["rb"][:, jsl], op=ALU.mult)
            s["y2sb"] = y2sb

        def emit_wy(h, c, on_scalar):
            # wy psum for channel-group c + copy into wy_sb (Pool/DVE for h0,
            # scalar for h1 tail where Act is idle)
            s = H[h]
            n0 = s["n0"]
            for j in range(2):
                jsl = slice(512 * j, 512 * (j + 1))
                wyp = pa.tile([IC, 512], f32, tag="y2a", bufs=2,
                              name=f"wyp{h}_{c}_{j}")
                _mm(nc, wyp[:], wwT_b[:, 128 * c:128 * (c + 1)],
                    s["y2sb"][:, jsl])
                dst = wy_sb[c][:, n0 + 512 * j:n0 + 512 * (j + 1)]
                if on_scalar:
                    nc.scalar.copy(dst, wyp[:])
                else:
                    nc.vector.tensor_copy(dst, wyp[:])

        def emit_sq(h, c, on_scalar):
            # sum_n wy^2 for (h, c) from wy_sb f16
            s = H[h]
            n0 = s["n0"]
            src = wy_sb[c][:, n0:n0 + NH]
            acc = bp.tile([128, 1], f32, tag=f"sq{h}{c}", name=f"sq{h}{c}")
            if on_scalar:
                sqt = ep.tile([128, 1024], f16, tag="sqs", bufs=2,
                              name=f"sqt{h}{c}")
                nc.scalar.activation(sqt[:], src, ACTF.Square, accum_out=acc[:])
            else:
                sqt = ep.tile([128, 1024], f16, tag="sqs", bufs=2,
                              name=f"sqt{h}{c}")
                nc.vector.tensor_tensor(sqt[:], src, src, op=ALU.mult)
                nc.vector.tensor_reduce(acc[:], sqt[:],
                                        mybir.AxisListType.X, ALU.add)
            sq_sum[(h, c)] = acc

        def emit_slin(h):
            acc = bp.tile([128, 1], f32, tag=f"slin{h}", name=f"slin{h}")
            nc.vector.tensor_reduce(acc[:], H[h]["y2sb"][:],
                                    mybir.AxisListType.X, ALU.add)
            slin[h] = acc

        # ---------------- main loop ----------------
        def emit_iter(h, m, slot=None):
            s = H[h]
            ft_cur = s["ft"]
            if m < 31:
                s["ft"] = emit_f(h, m + 1)
            if slot is not None:
                slot()
            expP = ep.tile([128, 1024], bf16, tag="exp", name=f"ex{h}_{m}")
            nc.scalar.activation(expP[:], ft_cur[:], ACTF.Exp)
            # 3-of-4 d-adds on DVE (bf16 2x), 1-of-4 on Pool (SBUF-only)
            a = 2 if (m & 3) == 3 else (m & 1)
            eng = nc.gpsimd if a == 2 else nc.vector
            acc = s["dacc"][a]
            if not s["dst"][a]:
                eng.tensor_copy(acc[:], expP[:])
                s["dst"][a] = True
            else:
                eng.tensor_tensor(acc[:], acc[:], expP[:], op=ALU.add)
            for j in range(2):
                _mm(nc, s["y2"][j][:], g_sb[:, 128 * m:128 * (m + 1)],
                    expP[:, 512 * j:512 * (j + 1)],
                    start=(m == 0), stop=(m == 31))

        # --- pre-loop projections (y chunk 0 dependent) ---
        emit_theta_block(0)
        emit_theta_block(1)
        emit_phi_block(0)
        emit_g_block(0)
        emit_phi_block(1)
        emit_g_block(1)

        h0_slots = {
            2: lambda: emit_phi_block(2),
            3: lambda: emit_g_block(2),
            4: lambda: emit_y_dma(2),
            6: lambda: emit_phi_block(3),
            7: lambda: emit_g_block(3),
            8: lambda: emit_theta_block(2),
            9: lambda: emit_theta_block(3),
            10: lambda: emit_y_dma(3),
            12: lambda: emit_phi_block(4),
            13: lambda: emit_g_block(4),
            16: lambda: emit_phi_block(5),
            17: lambda: emit_g_block(5),
            20: lambda: emit_phi_block(6),
            21: lambda: emit_g_block(6),
            24: lambda: emit_phi_block(7),
            25: lambda: emit_g_block(7),
        }
        h1_slots = {
            1: lambda: emit_dq(0),
            2: lambda: emit_rb(0),
            3: lambda: emit_y2norm(0),
            4: lambda: emit_wy(0, 0, on_scalar=False),
            5: lambda: emit_wy(0, 1, on_scalar=False),
            7: lambda: emit_sq(0, 0, on_scalar=False),
            9: lambda: emit_sq(0, 1, on_scalar=False),
            11: lambda: emit_slin(0),
        }

        with nc.allow_low_precision("bf16 softmax denominator accumulate"):
            begin_half(0)
            for m in range(32):
                emit_iter(0, m, h0_slots.get(m))
            begin_half(1)
            for m in range(32):
                emit_iter(1, m, h1_slots.get(m))

            # ---------------- tail: half 1 norm + wy + stats ----------
            emit_dq(1)
            emit_rb(1)
            emit_y2norm(1)
            emit_wy(1, 0, on_scalar=True)
            emit_wy(1, 1, on_scalar=True)
            emit_slin(1)
            emit_sq(1, 0, on_scalar=True)
            emit_sq(1, 1, on_scalar=True)

        # linear stats: sum_n wy = wwT.T @ (slin0 + slin1), exact f32 matmul
        s01 = bp.tile([128, 1], f32, tag="s01")
        nc.vector.tensor_tensor(s01[:], slin[0][:], slin[1][:], op=ALU.add)
        mps = pa.tile([IC, 512], f32, tag="y2a", bufs=2, name="meanps")
        for c in range(2):
            _mm(nc, mps[:, c:c + 1], w2[:, 128 * c:128 * (c + 1)], s01[:])

        packed = bp.tile([128, 4], f32, tag="packed")
        nc.vector.tensor_copy(packed[:, 0:2], mps[:, 0:2])
        for c in range(2):
            nc.vector.tensor_tensor(packed[:, 2 + c:3 + c], sq_sum[(0, c)][:],
                                    sq_sum[(1, c)][:], op=ALU.add)

        # ---------------- AllReduce of BN stats ----------------
        ar_in = dr.tile([128, 4], f32, name="ar_in")
        ar_out = dr.tile([128, 4], f32, name="ar_out")
        nc.sync.dma_start(ar_in[:], packed[:])
        nc.gpsimd.collective_compute(
            "AllReduce", ALU.add,
            replica_groups=[list(range(N_CORES))],
            ins=[ar_in.opt()], outs=[ar_out.opt()])
        gsb = bp.tile([128, 4], f32, tag="gsb")
        nc.sync.dma_start(gsb[:], ar_out[:])

        # ---------------- BN math + apply + residual ----------------
        for c in range(2):
            mean = bp.tile([128, 1], f32, tag=f"mean{c}")
            nc.vector.tensor_scalar(mean[:], gsb[:, c:c + 1],
                                    1.0 / CNT, None, ALU.mult)
            msq = bp.tile([128, 1], f32, tag=f"msq{c}")
            nc.vector.tensor_scalar(msq[:], gsb[:, 2 + c:3 + c],
                                    1.0 / CNT, None, ALU.mult)
            m2 = bp.tile([128, 1], f32, tag=f"m2{c}")
            nc.vector.tensor_tensor(m2[:], mean[:], mean[:], op=ALU.mult)
            var = bp.tile([128, 1], f32, tag=f"var{c}")
            nc.vector.tensor_tensor(var[:], msq[:], m2[:], op=ALU.subtract)
            varep = bp.tile([128, 1], f32, tag=f"varep{c}")
            nc.vector.tensor_scalar(varep[:], var[:], float(EPS), None, ALU.add)
            sd = bp.tile([128, 1], f32, tag=f"sd{c}")
            nc.scalar.activation(sd[:], varep[:], ACTF.Sqrt)
            rstd = bp.tile([128, 1], f32, tag=f"rstd{c}")
            nc.vector.reciprocal(rstd[:], sd[:])
            scale = bp.tile([128, 1], f32, tag=f"scale{c}")
            nc.vector.tensor_tensor(scale[:], gamma_t[c], rstd[:], op=ALU.mult)
            msc = bp.tile([128, 1], f32, tag=f"msc{c}")
            nc.vector.tensor_tensor(msc[:], mean[:], scale[:], op=ALU.mult)
            shift = bp.tile([128, 1], f32, tag=f"shift{c}")
            nc.vector.tensor_tensor(shift[:], beta_t[c], msc[:], op=ALU.subtract)

            out_t = mp.tile([128, NL], f32, tag=f"out{c}", bufs=1,
                            name=f"out{c}")
            for k in range(2):
                sl = slice(1024 * k, 1024 * (k + 1))
                nc.vector.affine_then_add(out_t[:, sl], wy_sb[c][:, sl],
                                          xl_t[c][:, sl], scale[:], shift[:])
                nc.sync.dma_start(out_d[128 * c:128 * (c + 1), sl],
                                  out_t[:, sl])


_NC_CACHE = None


def _get_nc():
    global _NC_CACHE
    if _NC_CACHE is None:
        _NC_CACHE = _build()
    return _NC_CACHE


def shard_inputs(inputs):
    x = np.ascontiguousarray(inputs["x"], dtype=np.float32).reshape(B, C, N)
    y = np.ascontiguousarray(inputs["y"], dtype=np.float32).reshape(B, C, N)
    dxwT = np.asarray(inputs["dx_w"]).T.astype(np.float32)
    dywT = np.asarray(inputs["dy_w"]).T.astype(np.float32)
    gwT = np.asarray(inputs["g_w"]).T.astype(np.float32)
    wwT = np.asarray(inputs["w_w"]).T.astype(np.float32)
    dxb = np.asarray(inputs["dx_b"], dtype=np.float32).reshape(IC, 1)
    gamma = np.asarray(inputs["bn_gamma"], dtype=np.float32).reshape(C, 1)
    beta = np.asarray(inputs["bn_beta"], dtype=np.float32).reshape(C, 1)
    # pack all small weights into two tensors (3 DMAs instead of 12)
    wpk = np.ascontiguousarray(
        np.concatenate([dxwT, dywT, gwT, gamma, beta], axis=1))   # [256, 386]
    wpk2 = np.ascontiguousarray(
        np.concatenate([wwT, dxb], axis=1))                        # [128, 257]

    in_maps = []
    for core in range(N_CORES):
        b, h = divmod(core, 2)
        in_maps.append({
            "xl": np.ascontiguousarray(x[b][:, h * NL:(h + 1) * NL]),
            "yl": y[b],
            "wpk": wpk, "wpk2": wpk2,
        })
    return in_maps


def run(inputs, **kw):
    """Run on hardware; returns (full_output, BassKernelResults)."""
    nc = _get_nc()
    in_maps = shard_inputs(inputs)
    r = run_bass_kernel_spmd(nc, in_maps, core_ids=list(range(N_CORES)), **kw)
    out = np.empty((B, C, N), np.float32)
    for core in range(N_CORES):
        b, h = divmod(core, 2)
        out[b][:, h * NL:(h + 1) * NL] = r.results[core]["out"]
    return out.reshape(B, C, HW, HW), r


def kernel(**inputs):
    out, _ = run(inputs)
    return out
